# revision 2
# baseline (speedup 1.0000x reference)
"""Peephole-LSTM Trainium2 kernel builder (per-core program, SPMD over 8 cores).

Each core owns NB=16 batch rows and runs the full T-step recurrence locally
(weights replicated). Per-core layout:

  Column groups: g in 0..3 <-> H-slice Hg = [256g, 256g+256).
  "c-layout" tiles [128, 256]: partition 32g+r (r<16) = (batch row r, Hg).
  PSUM main tile [128, 1280] per step (partitions 32g+r):
     cols [0:256)=peep_i|Hg [256:512)=peep_f|Hg       (bank 0)
          [512:768)=pre_g|Hg [768:1024)=pre_i|Hg      (bank 1)
          [1024:1280)=pre_f|Hg                        (bank 2)
  Matmuls: stationary = cT (bf16 [128, 128], col 16k+r = c[r, 128k+:]),
  moving = packed weights, 4-way PE column-tiling (one col-group per g).
  xW (input contribution + bias) is precomputed on-device into DRAM
  (weights-stationary GEMM) and injected into PSUM via identity matmuls.
  o/h are only computed once per row at t=lens[b]-1, after the loop, via
  indirect-DMA gathers from the c history in DRAM.
"""

import numpy as np
import ml_dtypes

import concourse.bass as bass
import concourse.bacc as bacc
import concourse.mybir as mybir
import concourse.tile as tile
from concourse.bass import ds

F32 = mybir.dt.float32
BF16 = mybir.dt.bfloat16
I32 = mybir.dt.int32
AF = mybir.ActivationFunctionType
BF = ml_dtypes.bfloat16

B, T_FULL, I_DIM, H = 128, 1024, 512, 1024
NB = 16              # batch rows per core
NG = 4               # column groups
HG = H // NG         # 256
KC = H // 128        # 8 k-chunks
IC = I_DIM // 128    # 4 input chunks
UPW = NG * 5 * HG    # 5120
XWW = 3 * HG         # 768


# ----------------------------------------------------------------- host packing

def pack_weights(U, P, W, P_o, bias):
    U4 = U.reshape(H, 4, NG, HG)     # [h, gate(i f g o), g, j]
    P2 = P.reshape(H, 2, NG, HG)     # [h, (pi pf), g, j]
    up = np.concatenate(
        [P2[:, 0], P2[:, 1], U4[:, 2], U4[:, 0], U4[:, 1]], axis=-1
    )  # [H, NG, 5*HG]
    up_pack = np.ascontiguousarray(up.reshape(KC, 128, UPW)).astype(BF)

    W4 = W.reshape(I_DIM, 4, NG, HG)
    w_re = np.concatenate([W4[:, 2], W4[:, 0], W4[:, 1]], axis=-1)
    w_pack = np.ascontiguousarray(w_re.reshape(IC, 128, NG * 3 * HG)).astype(BF)

    b4 = bias.reshape(4, NG, HG)
    b_re = np.concatenate([b4[2], b4[0], b4[1]], axis=-1)
    bias_pack = np.ascontiguousarray(b_re.reshape(1, NG * 3 * HG)).astype(BF)

    uo_pack = np.ascontiguousarray(U[:, 3 * H:].reshape(KC, 128, H)).astype(BF)
    po_pack = np.ascontiguousarray(P_o.reshape(KC, 128, H)).astype(BF)
    wo_pack = np.ascontiguousarray(W[:, 3 * H:].reshape(IC, 128, H)).astype(BF)
    biaso_pack = np.ascontiguousarray(bias[3 * H:].reshape(1, H)).astype(BF)
    return dict(up=up_pack, w=w_pack, biasifg=bias_pack, uo=uo_pack,
                po=po_pack, wo=wo_pack, biaso=biaso_pack)


def pack_consts():
    id16 = np.zeros((128, 16), dtype=BF)
    id16f = np.zeros((128, 16), dtype=np.float32)
    for g in range(NG):
        for j in range(16):
            id16[32 * g + j, j] = 1.0
            id16f[32 * g + j, j] = 1.0
    ones1 = np.ones((1, 128), dtype=BF)
    id128 = np.eye(128, dtype=np.float32)
    return dict(id16=id16, id16f=id16f, ones1=ones1, id128=id128)


def pack_core_inputs(x_core, lens_core, t_steps):
    t = t_steps
    x_tb = np.ascontiguousarray(
        x_core[:, :t, :].transpose(1, 0, 2)).reshape(t * NB, I_DIM)
    xT = np.ascontiguousarray(x_tb.T).reshape(IC, 128, t * NB).astype(BF)
    L = np.minimum(lens_core.astype(np.int64), t)
    x_f = np.ascontiguousarray(x_core[np.arange(NB), L - 1, :]).astype(BF)
    gidx = np.zeros((16, 8), np.int32)
    for r in range(NB):
        for g in range(NG):
            gidx[r, g] = int(L[r]) * 128 + 32 * g + r
            gidx[r, 4 + g] = (int(L[r]) - 1) * 128 + 32 * g + r
    return dict(xT=xT, x_f=x_f, gidx=gidx)


# ----------------------------------------------------------------- the program

def _phase_precompute(nc, tc, cst, t, xT_in, w_in, biasifg_in, xw_hist,
                      dbg_xwsb_out=None):
    nmt = (t * NB) // 128
    with (
        tc.tile_pool(name="wre", bufs=1) as wrep,
        tc.tile_pool(name="xt", bufs=3) as xtp,
        tc.tile_pool(name="xwsb", bufs=3) as xwsbp,
        tc.tile_pool(name="biassb", bufs=1) as biasp,
        tc.tile_pool(name="pspre", bufs=1, space="PSUM") as psprep,
    ):
        w_sb = wrep.tile([128, IC * 3 * H], BF16)
        for c in range(IC):
            nc.sync.dma_start(w_sb[:, c * 3 * H : (c + 1) * 3 * H], w_in[c, :, :])
        bias_sb = biasp.tile([1, 3 * H], BF16)
        nc.sync.dma_start(bias_sb[:], biasifg_in[:])

        for m in range(nmt):
            xt_t = xtp.tile([128, IC * 128], BF16)
            for c in range(IC):
                nc.sync.dma_start(
                    xt_t[:, c * 128 : (c + 1) * 128],
                    xT_in[c, :, ds(m * 128, 128)],
                )
            ps = psprep.tile([128, 3 * H], F32)
            for nb6 in range(6):
                cs = slice(512 * nb6, 512 * nb6 + 512)
                nc.tensor.matmul(
                    ps[:, cs], cst["ones1"][0:1, :], bias_sb[0:1, cs],
                    start=True, stop=False)
                for c in range(IC):
                    nc.tensor.matmul(
                        ps[:, cs],
                        xt_t[:, c * 128 : (c + 1) * 128],
                        w_sb[:, c * 3 * H + 512 * nb6 : c * 3 * H + 512 * nb6 + 512],
                        start=False, stop=(c == IC - 1))
            xw_sb = xwsbp.tile([128, 3 * H], BF16)
            if m % 2 == 0:
                nc.vector.tensor_copy(xw_sb[:], ps[:])
            else:
                nc.scalar.copy(xw_sb[:], ps[:])
            if dbg_xwsb_out is not None and m == 0:
                nc.sync.dma_start(dbg_xwsb_out[:], xw_sb[:])
            # scatter: sbuf [(s b), (g j)] -> dram rows 128s+32g+b
            for g in range(NG):
                nc.sync.dma_start(
                    xw_hist[ds(m * 8 * 128, 8 * 128), :].rearrange(
                        "(s gq) j -> s gq j", gq=128)[:, 32 * g : 32 * g + NB, :],
                    xw_sb[:, XWW * g : XWW * (g + 1)],
                )


def _phase_loop(nc, tc, cst, t, up_in, xw_hist, c_hist):
    id16 = cst["id16"]
    with (
        tc.tile_pool(name="upw", bufs=1) as upp,
        tc.tile_pool(name="xwblk", bufs=4) as xwblkp,
        tc.tile_pool(name="state", bufs=1) as statep,
        tc.tile_pool(name="eltw", bufs=2) as eltp,
        tc.tile_pool(name="psmain", bufs=1, space="PSUM") as psmp,
        tc.tile_pool(name="pstr", bufs=1, space="PSUM") as pstrp,
    ):
        up_sb = []
        for k in range(KC):
            u = upp.tile([128, UPW], BF16, tag=f"up{k}", name=f"up{k}")
            nc.sync.dma_start(u[:], up_in[k, :, :])
            up_sb.append(u)

        c_tiles = [statep.tile([128, HG], F32, tag="cA", name="cA"),
                   statep.tile([128, HG], F32, tag="cB", name="cB")]
        cT_tiles = [statep.tile([128, 128], BF16, tag="cTA", name="cTA"),
                    statep.tile([128, 128], BF16, tag="cTB", name="cTB")]
        nc.vector.memset(c_tiles[0][:], 0.0)
        nc.vector.memset(cT_tiles[0][:], 0.0)
        nc.sync.dma_start(c_hist[ds(0, 128), :], c_tiles[0][:])

        for step in range(t):
            c_cur = c_tiles[step % 2]
            c_nxt = c_tiles[(step + 1) % 2]
            cT_cur = cT_tiles[step % 2]
            cT_nxt = cT_tiles[(step + 1) % 2]

            xws = xwblkp.tile([128, XWW], BF16, tag="xws", name="xws")
            nc.sync.dma_start(xws[:], xw_hist[ds(step * 128, 128), :])

            ps = psmp.tile([128, 5 * HG], F32)
            # main matmuls, block-major (peep first), round-robin groups
            for (co, cw, st_k0, sp_k7) in [
                (0, 512, True, True),       # peep (bank0)
                (512, 512, True, False),    # pre_g + pre_i (bank1)
                (1024, 256, True, False),   # pre_f (bank2)
            ]:
                for k in range(KC):
                    for g in range(NG):
                        nc.tensor.matmul(
                            ps[32 * g : 32 * g + NB, co : co + cw],
                            cT_cur[:, 16 * k : 16 * k + 16],
                            up_sb[k][:, (UPW // NG) * g + co :
                                     (UPW // NG) * g + co + cw],
                            start=st_k0 and (k == 0),
                            stop=sp_k7 and (k == KC - 1),
                            tile_position=(0, 32 * g),
                        )
            # tanh(peep) -> tp bf16 (junk lanes harmless)
            tp = eltp.tile([128, 512], BF16, tag="tp", name="tp")
            nc.scalar.activation(tp[:], ps[:, 0:512], AF.Tanh)
            # xw injection: pre_g+pre_i [512:1024), pre_f [1024:1280)
            for g in range(NG):
                sl = slice(32 * g, 32 * g + NB)
                tpos = (32 * g, 32 * g)
                nc.tensor.matmul(
                    ps[sl, 512:1024], id16[sl, :], xws[sl, 0:512],
                    start=False, stop=False, tile_position=tpos,
                    skip_group_check=True)
                nc.tensor.matmul(
                    ps[sl, 1024:1280], id16[sl, :], xws[sl, 512:768],
                    start=False, stop=False, tile_position=tpos,
                    skip_group_check=True)
            # tanh(peep) injection into pre_i / pre_f
            for g in range(NG):
                sl = slice(32 * g, 32 * g + NB)
                tpos = (32 * g, 32 * g)
                nc.tensor.matmul(
                    ps[sl, 768:1024], id16[sl, :], tp[sl, 0:256],
                    start=False, stop=True, tile_position=tpos,
                    skip_group_check=True)
                nc.tensor.matmul(
                    ps[sl, 1024:1280], id16[sl, :], tp[sl, 256:512],
                    start=False, stop=True, tile_position=tpos,
                    skip_group_check=True)

            sig_if = eltp.tile([128, 512], F32, tag="sig", name="sig")
            nc.scalar.activation(sig_if[:], ps[:, 768:1280], AF.Sigmoid)
            tg = eltp.tile([128, 256], F32, tag="tg", name="tg")
            nc.scalar.activation(tg[:], ps[:, 512:768], AF.Tanh)

            t1 = eltp.tile([128, HG], F32, tag="t1", name="t1")
            t2 = eltp.tile([128, HG], F32, tag="t2", name="t2")
            nc.vector.tensor_mul(t1[:], sig_if[:, 0:256], tg[:])
            nc.vector.tensor_mul(t2[:], sig_if[:, 256:512], c_cur[:])
            nc.vector.tensor_add(c_nxt[:], t1[:], t2[:])

            for hh in range(2):
                pst = pstrp.tile([128, 128], F32, tag=f"pst{hh}",
                                 name=f"pst{hh}")
                nc.tensor.transpose(
                    pst[:, :], c_nxt[:, 128 * hh : 128 * hh + 128],
                    cst["id128"][:, :])
                nc.vector.tensor_copy(
                    cT_nxt[:].rearrange(
                        "p (g two r) -> p g two r", g=4, two=2)[:, :, hh, :],
                    pst[:].rearrange("p (g r) -> p g r", g=4)[:, :, 0:16],
                )
            nc.sync.dma_start(c_hist[ds((step + 1) * 128, 128), :], c_nxt[:])


def _phase_finalize(nc, tc, cst, uo_in, po_in, wo_in, biaso_in,
                    gidx_in, xf_in, c_hist, h_out):
    id16, id16f, ones1 = cst["id16"], cst["id16f"], cst["ones1"]
    with (
        tc.tile_pool(name="finw", bufs=1) as finwp,
        tc.tile_pool(name="fin", bufs=1) as finp,
        tc.tile_pool(name="psfin", bufs=1, space="PSUM") as psfp,
        tc.tile_pool(name="psfin2", bufs=1, space="PSUM") as psf2p,
        tc.tile_pool(name="pstf", bufs=1, space="PSUM") as pstfp,
    ):
        uo_sb = finwp.tile([128, KC * H], BF16, tag="uo", name="uo")
        po_sb = finwp.tile([128, KC * H], BF16, tag="po", name="po")
        wo_sb = finwp.tile([128, IC * H], BF16, tag="wo", name="wo")
        for k in range(KC):
            nc.sync.dma_start(uo_sb[:, k * H : (k + 1) * H], uo_in[k, :, :])
            nc.sync.dma_start(po_sb[:, k * H : (k + 1) * H], po_in[k, :, :])
        for c in range(IC):
            nc.sync.dma_start(wo_sb[:, c * H : (c + 1) * H], wo_in[c, :, :])
        bo_sb = finp.tile([1, H], BF16, tag="bo", name="bo")
        nc.sync.dma_start(bo_sb[:], biaso_in[:])
        gidx = finp.tile([16, 8], I32, tag="gidx", name="gidx")
        nc.sync.dma_start(gidx[:], gidx_in[:])
        xf_sb = finp.tile([NB, I_DIM], BF16, tag="xf", name="xf")
        nc.sync.dma_start(xf_sb[:], xf_in[:])

        cout_b = finp.tile([NB, H], F32, tag="cout", name="cout")
        cin_b = finp.tile([NB, H], F32, tag="cin", name="cin")
        for g in range(NG):
            nc.gpsimd.indirect_dma_start(
                out=cout_b[:, HG * g : HG * (g + 1)], out_offset=None,
                in_=c_hist[:],
                in_offset=bass.IndirectOffsetOnAxis(ap=gidx[:, g : g + 1], axis=0),
            )
            nc.gpsimd.indirect_dma_start(
                out=cin_b[:, HG * g : HG * (g + 1)], out_offset=None,
                in_=c_hist[:],
                in_offset=bass.IndirectOffsetOnAxis(ap=gidx[:, 4 + g : 5 + g], axis=0),
            )

        def transpose_to_bf16(src_b, nm, nchunk, ident, psum_dtype):
            dst = finp.tile([128, 16 * nchunk], BF16, tag=nm, name=nm)
            pstf = pstfp.tile([128, 128], psum_dtype, tag="pstf" + nm,
                              name="pstf" + nm)
            for k in range(nchunk):
                nc.tensor.transpose(
                    pstf[:, 16 * k : 16 * k + 16],
                    src_b[0:NB, 128 * k : 128 * k + 128],
                    ident[0:16, 0:16],
                )
            nc.vector.tensor_copy(dst[:], pstf[:, 0 : 16 * nchunk])
            return dst

        xtT = transpose_to_bf16(xf_sb, "xtT", IC, id16, BF16)
        cinT = transpose_to_bf16(cin_b, "cinT", KC, id16f, F32)
        coutT = transpose_to_bf16(cout_b, "coutT", KC, id16f, F32)

        ps_o = psfp.tile([NB, H], F32)
        ps_po = psf2p.tile([NB, H], F32)
        for half in range(2):
            cs = slice(512 * half, 512 * half + 512)
            nc.tensor.matmul(ps_o[:, cs], ones1[0:1, 0:NB], bo_sb[0:1, cs],
                             start=True, stop=False)
            for c in range(IC):
                nc.tensor.matmul(
                    ps_o[:, cs], xtT[:, 16 * c : 16 * c + 16],
                    wo_sb[:, c * H + 512 * half : c * H + 512 * half + 512],
                    start=False, stop=False)
            for k in range(KC):
                nc.tensor.matmul(
                    ps_o[:, cs], cinT[:, 16 * k : 16 * k + 16],
                    uo_sb[:, k * H + 512 * half : k * H + 512 * half + 512],
                    start=False, stop=False)
                nc.tensor.matmul(
                    ps_po[:, cs], coutT[:, 16 * k : 16 * k + 16],
                    po_sb[:, k * H + 512 * half : k * H + 512 * half + 512],
                    start=(k == 0), stop=(k == KC - 1))
        tpo = finp.tile([NB, H], BF16, tag="tpo", name="tpo")
        nc.scalar.activation(tpo[:], ps_po[:], AF.Tanh)
        for half in range(2):
            cs = slice(512 * half, 512 * half + 512)
            nc.tensor.matmul(ps_o[:, cs], id16[0:NB, :], tpo[:, cs],
                             start=False, stop=True, skip_group_check=True)
        o_sb = finp.tile([NB, H], F32, tag="osb", name="osb")
        nc.scalar.activation(o_sb[:], ps_o[:], AF.Sigmoid)
        tanc = finp.tile([NB, H], F32, tag="tanc", name="tanc")
        nc.scalar.activation(tanc[:], cout_b[:], AF.Tanh)
        h_sb = finp.tile([NB, H], F32, tag="hsb", name="hsb")
        nc.vector.tensor_mul(h_sb[:], o_sb[:], tanc[:])
        nc.sync.dma_start(h_out[:], h_sb[:])


def build_program(t_steps=T_FULL, parts=("pre", "loop", "fin"),
                  c_hist_out=False):
    t = t_steps
    assert (t * NB) % 128 == 0

    nc = bacc.Bacc(None, target_bir_lowering=False, debug=False)
    dp = nc.declare_dram_parameter
    up_in = dp("up", [KC, 128, UPW], BF16, isOutput=False)
    w_in = dp("w", [IC, 128, NG * 3 * HG], BF16, isOutput=False)
    biasifg_in = dp("biasifg", [1, NG * 3 * HG], BF16, isOutput=False)
    uo_in = dp("uo", [KC, 128, H], BF16, isOutput=False)
    po_in = dp("po", [KC, 128, H], BF16, isOutput=False)
    wo_in = dp("wo", [IC, 128, H], BF16, isOutput=False)
    biaso_in = dp("biaso", [1, H], BF16, isOutput=False)
    xT_in = dp("xT", [IC, 128, t * NB], BF16, isOutput=False)
    xf_in = dp("x_f", [NB, I_DIM], BF16, isOutput=False)
    gidx_in = dp("gidx", [16, 8], I32, isOutput=False)
    id16_in = dp("id16", [128, 16], BF16, isOutput=False)
    id16f_in = dp("id16f", [128, 16], F32, isOutput=False)
    id128_in = dp("id128", [128, 128], F32, isOutput=False)
    ones1_in = dp("ones1", [1, 128], BF16, isOutput=False)
    h_out = dp("h_out", [NB, H], F32, isOutput=True)

    xw_hist = nc.dram_tensor("xw_hist", [t * 128, XWW], BF16)
    if c_hist_out:
        c_hist = dp("c_hist", [(t + 1) * 128, HG], F32, isOutput=True)
    else:
        c_hist = nc.dram_tensor("c_hist", [(t + 1) * 128, HG], F32)

    with tile.TileContext(nc) as tc:
        with tc.tile_pool(name="const", bufs=1) as constp:
            id16 = constp.tile([128, 16], BF16)
            nc.sync.dma_start(id16[:], id16_in[:])
            id16f = constp.tile([128, 16], F32)
            nc.sync.dma_start(id16f[:], id16f_in[:])
            ones1 = constp.tile([1, 128], BF16)
            nc.sync.dma_start(ones1[:], ones1_in[:])
            id128 = constp.tile([128, 128], F32)
            nc.sync.dma_start(id128[:], id128_in[:])
            cst = dict(id16=id16, id16f=id16f, ones1=ones1, id128=id128)

            if "pre" in parts:
                _phase_precompute(nc, tc, cst, t, xT_in, w_in, biasifg_in,
                                  xw_hist)
            if "loop" in parts:
                _phase_loop(nc, tc, cst, t, up_in, xw_hist, c_hist)
            if "fin" in parts:
                _phase_finalize(nc, tc, cst, uo_in, po_in, wo_in, biaso_in,
                                gidx_in, xf_in, c_hist, h_out)
            else:
                with tc.tile_pool(name="dummy", bufs=1) as dummyp:
                    hz = dummyp.tile([NB, H], F32)
                    nc.vector.memset(hz[:], 0.0)
                    nc.sync.dma_start(h_out[:], hz[:])

    nc.compile()
    return nc


# ------------------------------------------------------- full host-side kernel

def make_in_maps(inputs, t_steps):
    x = np.asarray(inputs["x"], np.float32)
    lens = np.asarray(inputs["lens"]).astype(np.int64)
    wp = pack_weights(np.asarray(inputs["U"], np.float32),
                      np.asarray(inputs["P"], np.float32),
                      np.asarray(inputs["W"], np.float32),
                      np.asarray(inputs["P_o"], np.float32),
                      np.asarray(inputs["bias"], np.float32))
    cp = pack_consts()
    shared = {**wp, **cp}
    in_maps = []
    for core in range(8):
        sl = slice(core * NB, (core + 1) * NB)
        ci = pack_core_inputs(x[sl], lens[sl], t_steps)
        in_maps.append({**shared, **ci})
    return in_maps


def _ensure_axon_ntff_hook():
    """Provide antenv.axon_hooks (missing in this image) so BASS_TRACE works."""
    import sys, types, contextlib, ctypes, glob
    try:
        from antenv import axon_hooks  # noqa: F401
        return
    except ImportError:
        pass
    import antenv
    so_candidates = glob.glob("/opt/axon/libaxon_pjrt.so")
    if not so_candidates:
        return
    try:
        lib = ctypes.CDLL(so_candidates[0])
    except OSError:
        return
    if not hasattr(lib, "axon_start_nrt_profile"):
        return
    lib.axon_start_nrt_profile.argtypes = [
        ctypes.POINTER(ctypes.c_int64), ctypes.c_size_t]
    lib.axon_start_nrt_profile.restype = ctypes.c_int64
    lib.axon_stop_nrt_profile.argtypes = [ctypes.c_char_p]
    lib.axon_stop_nrt_profile.restype = ctypes.c_int64

    @contextlib.contextmanager
    def _hook(output_dir, device_ids):
        import jax
        jax.devices()
        if device_ids:
            ids = (ctypes.c_int64 * len(device_ids))(*device_ids)
            rc = lib.axon_start_nrt_profile(ids, len(device_ids))
        else:
            rc = lib.axon_start_nrt_profile(None, 0)
        if rc != 0:
            raise RuntimeError(f"axon_start_nrt_profile rc={rc}")
        try:
            yield
        finally:
            n = lib.axon_stop_nrt_profile(str(output_dir).encode())
            if n <= 0:
                print(f"ntff profile: rc={n} (no files?)")

    mod = types.ModuleType("antenv.axon_hooks")
    _holder = {"h": _hook}
    mod.set_axon_ntff_profile_hook = lambda h: _holder.__setitem__("h", h)
    mod.get_axon_ntff_profile_hook = lambda: _holder.get("h")
    sys.modules["antenv.axon_hooks"] = mod
    antenv.axon_hooks = mod


def run(inputs, t_steps=T_FULL, trace=False, parts=("pre", "loop", "fin"),
        c_hist_out=False, nc_cache={}):
    if trace:
        _ensure_axon_ntff_hook()
    from concourse.bass_utils import run_bass_kernel_spmd

    in_maps = make_in_maps(inputs, t_steps)
    key = (t_steps, parts, c_hist_out)
    if key not in nc_cache:
        nc_cache[key] = build_program(t_steps, parts, c_hist_out)
    nc = nc_cache[key]

    res = run_bass_kernel_spmd(nc, in_maps, list(range(8)), trace=trace)
    h = np.concatenate([res.results[i]["h_out"] for i in range(8)], axis=0)
    return h.astype(np.float32), res


# ======================================================================
# Public entry point: full inputs in, full output out.
# ======================================================================

LAST_EXEC_NS = None


def kernel(**inputs):
    """Peephole-LSTM forward; returns h at t=lens-1 for each row: [B, H] f32."""
    global LAST_EXEC_NS
    import os
    trace = bool(os.environ.get("BASS_TRACE"))
    h, res = run(inputs, t_steps=T_FULL, trace=trace)
    if res.exec_time_ns is not None:
        LAST_EXEC_NS = res.exec_time_ns
    return h

